# revision 29
# baseline (speedup 1.0000x reference)
"""Trainium2 Bass kernel for nn_CausalSelfAttention_59253368815644.

Sharding: 8 cores = 2 (batch) x 4 (head groups of 4 heads). Each core
computes qkv projection + rms-norm + rotary (transposed layout, PE-assisted
cross-partition reductions), KEY_OFFSET band shift (free-axis DMA shifts),
doc-masked causal attention (doc mask fused into the score matmul through
one-hot augmented contraction rows; causal via affine_select on boundary
tiles; softmax without max-subtraction -- scores are bounded by
attn_scale*HD), gated value embedding, attention output gate, and a partial
output projection over its 256 head-dim columns. Host sums 4 partials per
batch element.
"""
import sys

sys.path.insert(0, "/opt/trn_rl_repo")

from contextlib import ExitStack

import ml_dtypes
import numpy as np


import concourse.bass as bass
import concourse.tile as tile
from concourse import bacc, mybir
from concourse._compat import with_exitstack
from concourse.bass_utils import run_bass_kernel_spmd

F32 = mybir.dt.float32
F32R = mybir.dt.float32r
BF16 = mybir.dt.bfloat16
AF = mybir.ActivationFunctionType

B, T, D, H, HD = 2, 2048, 1024, 16, 64
EPS = 1.1920929e-07
VE_GATE_SCALE = 2.0
NHEADS = 4          # heads per core
HGROUPS = 4
NCHUNK = D // 128   # 8 contraction chunks
TTILE = 512
NTT = T // TTILE
BIG = 30.0          # mask exponent after exp-scale
NDOC = 8
AUG = NDOC + 1
QR = 64 + AUG       # 73 partitions for Q^/K^


def r32(a):
    return a  # operands are float32r-typed


def pbcast(row_ap, nparts):
    """Broadcast a [1, ...] AP along partitions."""
    return bass.AP(tensor=row_ap.tensor, offset=row_ap.offset,
                   ap=[[0, nparts]] + [list(d) for d in row_ap.ap[1:]])


def build_spans(segs):
    """Greedy partition of [0,T) into q-spans (len 256..512 where possible),
    preferring doc-boundary ends. Returns [(a, b, ks)] with ks = start of
    the doc containing a."""
    bounds = [e for (_, e) in segs]
    spans = []
    a = 0
    while a < T:
        cands = [e for e in bounds if a < e <= a + 512]
        end = None
        if cands:
            mx = max(cands)
            if mx - a >= 256 or mx == T:
                end = mx
        if end is None:
            end = min(a + 512, T)
        if end % 2 != 0 and end < T:
            end += 1
        ks = max((s for (s, _) in segs if s <= a), default=0)
        spans.append((a, end, ks))
        a = end
    out = []
    for (a, b, ks) in spans:
        ka0 = (ks // 128) * 128
        kts = []
        ka = ka0
        while ka < b:
            kn = min(128, b - ka)
            kts.append((ka, kn, (ka + kn) > a))
            ka += 128
        out.append((a, b, kts))
    return out


@with_exitstack
def build_kernel(ctx: ExitStack, tc: tile.TileContext, dr, spans, alpha):
    nc = tc.nc

    const = ctx.enter_context(tc.tile_pool(name="const", bufs=1))
    persist = ctx.enter_context(tc.tile_pool(name="persist", bufs=1))

    # Queue layout at startup: scalar carries the weights needed by the
    # first matmuls; sync carries x tiles 0/2; gpsimd carries rotary
    # tables + x tiles 1/3 + (late) phase-2 aug rows. Nothing tiny or
    # phase-2-only may ride ahead of the first x/w loads.
    wqk = const.tile([128, NCHUNK, 512], BF16)
    nc.sync.dma_start(wqk[:],
                      dr["wqk"][:].rearrange("p (c e) -> p c e", e=512))
    wv = const.tile([128, NCHUNK, 260], BF16)
    nc.scalar.dma_start(wv[:],
                        dr["wv"][:].rearrange("p (c e) -> p c e", e=260))
    wga = const.tile([128, NCHUNK, 34], BF16)
    nc.scalar.dma_start(wga[:],
                        dr["wga"][:].rearrange("p (c e) -> p c e", e=34))
    cdup = const.tile([128, T], BF16)
    nc.gpsimd.dma_start(cdup[:], dr["cdup"][:])
    s2dup = const.tile([128, T], BF16)
    nc.gpsimd.dma_start(s2dup[:], dr["s2dup"][:])
    ones2 = const.tile([128, 2], F32R)
    nc.scalar.dma_start(ones2[:], dr["ones2"][:])
    e2f = const.tile([8, 128], F32R)
    nc.scalar.dma_start(e2f[:], dr["e2f"][:])
    epsb = const.tile([8, 1], F32)
    nc.vector.memset(epsb[:], EPS)

    # Pre-load the one ACT table covering every function this kernel uses
    # (ln, exp, square, copy = natural_log_exp_and_others, id 6). The
    # insert_act_table_loads fixpoint then sees all activations covered and
    # inserts no further 1.3us table swaps.
    nc.scalar.add_instruction(mybir.InstLoadActFuncSet(
        name=nc.scalar.bass.get_next_instruction_name(),
        act_func_set_id=6, ins=[], outs=[]))

    Qh = persist.tile([QR, NHEADS, T], BF16)
    Kh = persist.tile([QR, NHEADS, T], BF16)
    Vh = persist.tile([128, T // 128, NHEADS, 65], BF16)
    # ones column (o=64) for the softmax denominator: memset the whole
    # tile -- phase 1 overwrites o=0:64, so only the ones column survives.
    # (A strided single-column memset fails walrus ISA checks; a tiny-
    # descriptor DMA broadcast clogs every DMA ring for ~55us.)
    nc.vector.memset(
        Vh[:].rearrange("p k h o -> p (k h o)").bitcast(mybir.dt.uint16),
        0x3F80)  # 1.0 in bf16 bits
    agrow0 = persist.tile([2, T], F32)
    agrow1 = persist.tile([2, T], F32)
    agrowp = [agrow0, agrow1]

    # =========== Phase 1 ===========
    with tc.tile_pool(name="p1x", bufs=2) as xpool, \
         tc.tile_pool(name="p1qk", bufs=4, space="PSUM") as qkps_pool, \
         tc.tile_pool(name="p1ss", bufs=1, space="PSUM") as ssps_pool, \
         tc.tile_pool(name="p1v", bufs=2, space="PSUM") as vps_pool, \
         tc.tile_pool(name="p1rb", bufs=1, space="PSUM") as rbps_pool, \
         tc.tile_pool(name="p1sb", bufs=2) as sb_pool, \
         tc.tile_pool(name="p1sm", bufs=2) as sm_pool, \
         tc.tile_pool(name="p1n", bufs=1) as n1_pool:
        for tt in range(NTT):
            t0 = tt * TTILE
            xsb = xpool.tile([128, NCHUNK, TTILE], BF16, tag="x")
            nc.sync.dma_start(
                xsb[:],
                dr["xT"][:, tt * NCHUNK * TTILE:(tt + 1) * NCHUNK * TTILE]
                .rearrange("p (c t) -> p c t", t=TTILE))
            if tt == 3:
                # phase-2 aug rows: issue only after every x tile is queued
                qaug = dr["qaug"]
                kaug = dr["kaug"]
                nc.gpsimd.dma_start(
                    Qh[64:QR, :, :],
                    bass.AP(tensor=qaug.tensor, offset=qaug.offset,
                            ap=[[T, AUG], [0, NHEADS], [1, T]]))
                nc.gpsimd.dma_start(
                    Kh[64:QR, :, :],
                    bass.AP(tensor=kaug.tensor, offset=kaug.offset,
                            ap=[[T, AUG], [0, NHEADS], [1, T]]))

            # attn-gate matmul: head pairs land at partition bases 0 and
            # 32 (engine partition windows must be 32-aligned so phase 2
            # can read per-pair slices)
            agps = vps_pool.tile([34, TTILE], F32, tag="v")
            for c in range(NCHUNK):
                nc.tensor.matmul(agps[:], r32(wga[:, c, :]), r32(xsb[:, c, :]),
                                 start=(c == 0), stop=(c == NCHUNK - 1))
            # sigmoid(z) = 1/(1+exp(-z)) -- keeps ACT on the ln/exp table.
            # ACT shifts the pair at base 32 down to base 0 so the DVE ops
            # and phase-2 reads stay base-aligned.
            for pr in (0, 1):
                age = n1_pool.tile([2, TTILE], F32, tag=f"age{pr}")
                nc.scalar.activation(out=age[:],
                                     in_=agps[32 * pr:32 * pr + 2, :],
                                     func=AF.Exp, scale=-1.0)
                nc.vector.tensor_scalar_add(age[:], age[:], 1.0)
                nc.vector.reciprocal_approx_fast(
                    out=agrowp[pr][:, t0:t0 + TTILE], in_=age[:])

            qks = []
            sqs = []
            # stage A: all qk matmuls
            for blk in range(NHEADS):
                qk = qkps_pool.tile([128, TTILE], F32, tag="qk")
                qks.append(qk)
                for c in range(NCHUNK):
                    nc.tensor.matmul(
                        qk[:], r32(wqk[:, c, blk * 128:(blk + 1) * 128]),
                        r32(xsb[:, c, :]),
                        start=(c == 0), stop=(c == NCHUNK - 1))
            # stage B: squares (in every ACT table)
            for blk in range(NHEADS):
                sq = n1_pool.tile([128, TTILE], F32R, tag="sq")
                sqs.append(sq)
                nc.scalar.activation(out=sq[:], in_=qks[blk][:],
                                     func=AF.Square, scale=1.0)
            # stage C: partition sums via PE
            sss = []
            for blk in range(NHEADS):
                ss = ssps_pool.tile([2, TTILE], F32, tag="ss")
                sss.append(ss)
                nc.tensor.matmul(ss[:], r32(ones2[:]), r32(sqs[blk][:]),
                                 start=True, stop=True)
            # stage D: rstd = exp(-0.5*ln(ms+eps)) -- ln/exp only, no
            # sqrt-table swap and no DVE reciprocal chain. Separate 2-row
            # tiles: matmul operands need base partition 0/32/64.
            rstds = []
            for blk in range(NHEADS):
                lms = n1_pool.tile([2, TTILE], F32, tag="lms")
                nc.scalar.activation(out=lms[:], in_=sss[blk][:],
                                     func=AF.Ln, scale=1.0 / HD,
                                     bias=epsb[0:2, :])
                rstd = n1_pool.tile([2, TTILE], F32R, tag=f"rstd{blk}")
                nc.scalar.activation(out=rstd[:], in_=lms[:],
                                     func=AF.Exp, scale=-0.5)
                rstds.append(rstd)
            for blk in range(NHEADS):
                # rstd broadcast to 128 partitions (f32r PE matmul)
                rbps = rbps_pool.tile([128, TTILE], F32, tag="rb")
                nc.tensor.matmul(rbps[:], e2f[0:2, :], rstds[blk][:],
                                 start=True, stop=True)
                qkb = sb_pool.tile([128, TTILE], BF16, tag="qkb")
                nc.scalar.activation(out=qkb[:], in_=qks[blk][:],
                                     func=AF.Copy, scale=1.0)
                A = sb_pool.tile([128, TTILE], BF16, tag="A")
                nc.vector.tensor_mul(A[:], qkb[:], cdup[:, t0:t0 + TTILE])
                Bt = sb_pool.tile([128, TTILE], BF16, tag="B")
                nc.vector.tensor_mul(Bt[:], qkb[:], s2dup[:, t0:t0 + TTILE])
                Bs = sb_pool.tile([128, TTILE], BF16, tag="Bs")
                nc.vector.stream_shuffle(Bs[:], Bt[:],
                                         mask=[g ^ 16 for g in range(32)])
                nc.vector.tensor_add(A[:], A[:], Bs[:])
                rot = sb_pool.tile([128, TTILE], BF16, tag="rot")
                nc.vector.tensor_mul(rot[:], A[:], rbps[:])
                h = blk
                nc.sync.dma_start(Qh[0:64, h, t0:t0 + TTILE], rot[0:64, :])
                nc.sync.dma_start(Kh[0:32, h, t0:t0 + TTILE], rot[64:96, :])
                w = TTILE if t0 + TTILE < T else TTILE - 1
                nc.sync.dma_start(Kh[32:64, h, t0 + 1:t0 + 1 + w],
                                  rot[96:128, 0:w])
                if t0 == 0:
                    nc.sync.dma_start(Kh[32:64, h, 0:1], rot[96:128, 0:1])

            for sub in range(TTILE // 128):
                st = t0 + sub * 128
                vps = vps_pool.tile([128, 260], F32, tag="v")
                for c in range(NCHUNK):
                    nc.tensor.matmul(
                        vps[:], r32(xsb[:, c, sub * 128:(sub + 1) * 128]),
                        r32(wv[:, c, :]),
                        start=(c == 0), stop=(c == NCHUNK - 1))
                g = sm_pool.tile([128, NHEADS], F32, tag="g")
                nc.scalar.activation(out=g[:], in_=vps[:, 256:260],
                                     func=AF.Exp, scale=-1.0)
                nc.vector.tensor_scalar_add(g[:], g[:], 1.0)
                nc.vector.reciprocal_approx_fast(out=g[:], in_=g[:])
                vesb = n1_pool.tile([128, 256], BF16, tag="ve")
                nc.scalar.dma_start(vesb[:], dr["ve2"][st:st + 128, :])
                gap = g[:]
                gb = bass.AP(tensor=gap.tensor, offset=gap.offset,
                             ap=[list(gap.ap[0]), [1, NHEADS], [0, HD]])
                tmp = n1_pool.tile([128, 256], F32, tag="vtmp")
                nc.gpsimd.tensor_mul(
                    tmp[:].rearrange("p (h d) -> p h d", h=NHEADS),
                    vesb[:].rearrange("p (h d) -> p h d", h=NHEADS), gb)
                nc.vector.tensor_add(
                    Vh[:, st // 128, :, 0:64],
                    vps[:, 0:256].rearrange("p (h d) -> p h d", h=NHEADS),
                    tmp[:].rearrange("p (h d) -> p h d", h=NHEADS))

    # =========== Phase 2 ===========
    ypool = ctx.enter_context(tc.tile_pool(name="ylate", bufs=1))
    y01 = ypool.tile([128, T], F32R)
    y23 = ypool.tile([128, T], F32R)
    wpool = ctx.enter_context(tc.tile_pool(name="p3w", bufs=1))
    wo = wpool.tile([128, 2, 1024], F32R)
    nc.gpsimd.dma_start(wo[:],
                        dr["wo"][:].rearrange("p (c e) -> p c e", e=1024))
    # Scores-first emission per head: the PE queue is in-order, so putting
    # every score matmul of a head ahead of its PV matmuls hides the
    # exp+affine_select latency (head-of-line blocking otherwise stalls PE
    # on every k-tile). Tails run per head-PAIR so a pair's softmax
    # normalize/gate work overlaps the other pair's matmuls, with yps
    # holding only 2 PSUM banks.
    with tc.tile_pool(name="p2s", bufs=3, space="PSUM") as sps_pool, \
         tc.tile_pool(name="p2y", bufs=1, space="PSUM") as yps_pool, \
         tc.tile_pool(name="p2b", bufs=1, space="PSUM") as bps_pool, \
         tc.tile_pool(name="p3ps", bufs=2, space="PSUM") as ops_pool, \
         tc.tile_pool(name="p3sb", bufs=3) as osb_pool, \
         tc.tile_pool(name="p2p", bufs=6) as pt_pool, \
         tc.tile_pool(name="p2sc", bufs=1) as sc_pool:

        def emit_oproj(ti):
            # o-projection for token tile ti; interleaved so PE chews on it
            # whenever attention stalls on exp/affine_select/tails
            tt0 = ti * 128
            for eh in range(2):
                ops = ops_pool.tile([128, 512], F32, tag="o")
                nc.tensor.matmul(ops[:], r32(y01[:, tt0:tt0 + 128]),
                                 r32(wo[:, 0, eh * 512:(eh + 1) * 512]),
                                 start=True, stop=False)
                nc.tensor.matmul(ops[:], r32(y23[:, tt0:tt0 + 128]),
                                 r32(wo[:, 1, eh * 512:(eh + 1) * 512]),
                                 start=False, stop=True)
                osb = osb_pool.tile([128, 512], F32, tag="osb")
                if eh == 0:
                    nc.scalar.activation(out=osb[:], in_=ops[:], func=AF.Copy,
                                         scale=1.0)
                else:
                    nc.vector.tensor_copy(osb[:], ops[:])
                nc.sync.dma_start(
                    dr["out"][tt0:tt0 + 128, eh * 512:(eh + 1) * 512], osb[:])

        next_ti = 0
        for (a, b_, kts) in spans:
            N = b_ - a
            nk = len(kts)
            for pr, ytile in ((0, y01), (1, y23)):
                gb0 = 32 * pr
                ypss = []
                for hh in (0, 1):
                    h = 2 * pr + hh
                    yps = yps_pool.tile([65, 512], F32, tag=f"y{hh}")
                    ypss.append(yps)
                    pts = []
                    for ki, (ka, kn, causal) in enumerate(kts):
                        w0 = max(0, ka - a)
                        sps = sps_pool.tile([128, 512], F32, tag="s")
                        nc.tensor.matmul(sps[0:kn, w0:N],
                                         r32(Kh[:, h, ka:ka + kn]),
                                         r32(Qh[:, h, a + w0:b_]),
                                         start=True, stop=True)
                        pt = pt_pool.tile([128, 512], BF16, tag="p")
                        pts.append(pt)
                        nc.scalar.activation(out=pt[0:kn, w0:N],
                                             in_=sps[0:kn, w0:N],
                                             func=AF.Exp, scale=alpha)
                        if causal:
                            bw = min(N, ka + kn - a) - w0
                            if bw > 0:
                                nc.gpsimd.affine_select(
                                    out=pt[0:kn, w0:w0 + bw],
                                    in_=pt[0:kn, w0:w0 + bw],
                                    compare_op=mybir.AluOpType.is_ge,
                                    fill=0.0, base=a + w0 - ka,
                                    pattern=[[1, bw]], channel_multiplier=-1)
                    for ki, (ka, kn, causal) in enumerate(kts):
                        w0 = max(0, ka - a)
                        nc.tensor.matmul(yps[:, w0:N],
                                         r32(Vh[0:kn, ka // 128, h, :]),
                                         r32(pts[ki][0:kn, w0:N]),
                                         start=(ki == 0), stop=(ki == nk - 1))
                l2 = sc_pool.tile([2, 512], F32, tag=f"l2_{pr}")
                for hh in (0, 1):
                    l1 = sc_pool.tile([1, 512], F32, tag=f"l1_{pr}{hh}")
                    nc.vector.tensor_copy(l1[:, 0:N], ypss[hh][64:65, 0:N])
                    nc.sync.dma_start(l2[hh:hh + 1, 0:N], l1[:, 0:N])
                rl2 = sc_pool.tile([2, 512], F32, tag=f"rl{pr}")
                nc.vector.reciprocal_approx_fast(out=rl2[:, 0:N],
                                                 in_=l2[:, 0:N])
                sc2 = sc_pool.tile([2, 512], F32R, tag=f"sc{pr}")
                nc.vector.tensor_mul(sc2[:, 0:N], rl2[:, 0:N],
                                     agrowp[pr][:, a:b_])
                sbc = bps_pool.tile([128, 512], F32, tag="sbc")
                nc.tensor.matmul(sbc[:, 0:N], r32(e2f[0:2, :]),
                                 r32(sc2[:, 0:N]), start=True, stop=True)
                yy = sc_pool.tile([128, 512], F32, tag=f"yy{pr}")
                nc.scalar.activation(out=yy[0:64, 0:N],
                                     in_=ypss[0][0:64, 0:N],
                                     func=AF.Copy, scale=1.0)
                nc.vector.tensor_copy(yy[64:128, 0:N], ypss[1][0:64, 0:N])
                nc.vector.tensor_mul(ytile[:, a:b_], yy[:, 0:N], sbc[:, 0:N])
            while (next_ti + 1) * 128 <= b_:
                emit_oproj(next_ti)
                next_ti += 1



_CACHE = {}
TRACE = False       # set by test harness to capture an NTFF profile
LAST_RESULT = None  # BassKernelResults of the most recent run


def _get_program(key, spans, alpha):
    if key in _CACHE:
        return _CACHE[key]
    nc = bacc.Bacc("TRN2", target_bir_lowering=False, debug=False)
    dr = {}

    def di(name, shape, dt=F32):
        dr[name] = nc.dram_tensor(name, shape, dt, kind="ExternalInput").ap()

    di("xT", [128, NCHUNK * T], BF16)
    di("ve2", [T, 256], BF16)
    di("wqk", [128, NCHUNK * 512], BF16)
    di("wv", [128, NCHUNK * 260], BF16)
    di("wga", [128, NCHUNK * 34], BF16)
    di("wo", [128, 2 * 1024], F32R)
    di("cdup", [128, T], BF16)
    di("s2dup", [128, T], BF16)
    di("qaug", [AUG, T], BF16)
    di("kaug", [AUG, T], BF16)
    di("ones2", [128, 2], F32R)
    di("e2f", [8, 128], F32R)
    dr["out"] = nc.dram_tensor("out", [T, D], F32, kind="ExternalOutput").ap()
    with tile.TileContext(nc) as tc:
        build_kernel(tc, dr, spans, alpha)
    nc.compile()
    _CACHE[key] = nc
    return nc


def kernel(x, ve, sa_lambdas, cos, sin, qkvo_w, attn_gate_w, ve_gate_w,
           attn_scale, docs):
    x = np.asarray(x, dtype=np.float32)
    ve = np.asarray(ve, dtype=np.float32)
    sa_lambdas = np.asarray(sa_lambdas, dtype=np.float32)
    cos = np.asarray(cos, dtype=np.float32)
    sin = np.asarray(sin, dtype=np.float32)
    qkvo_w = np.asarray(qkvo_w, dtype=np.float32)
    attn_gate_w = np.asarray(attn_gate_w, dtype=np.float32)
    ve_gate_w = np.asarray(ve_gate_w, dtype=np.float32)
    docs = np.asarray(docs, dtype=np.int32)
    alpha = float(np.asarray(attn_scale))

    segs = []
    s = 0
    for t in range(1, T + 1):
        if t == T or docs[t] != docs[t - 1]:
            segs.append((s, t))
            s = t
    spans = build_spans(segs)
    nc = _get_program((tuple(segs), alpha), spans, alpha)

    lam0, lam1 = float(sa_lambdas[0]), float(sa_lambdas[1])

    cosT = np.ascontiguousarray(cos.T)
    sinT = np.ascontiguousarray(sin.T)
    cblk = np.concatenate([cosT[0:16], cosT[0:16], cosT[16:32], cosT[16:32]],
                          axis=0)
    sblk = np.concatenate([-sinT[0:16], sinT[0:16], -sinT[16:32],
                           sinT[16:32]], axis=0)
    cdup = np.tile(cblk, (2, 1)).astype(ml_dtypes.bfloat16)
    s2dup = np.tile(sblk, (2, 1)).astype(ml_dtypes.bfloat16)
    onehot = (docs[None, :] == np.arange(NDOC)[:, None]).astype(np.float32)
    kaug = np.concatenate([onehot, np.ones((1, T), np.float32)],
                          axis=0).astype(ml_dtypes.bfloat16)
    qaug = np.concatenate(
        [(BIG / alpha) * onehot, -(BIG / alpha) * np.ones((1, T), np.float32)],
        axis=0).astype(ml_dtypes.bfloat16)
    ones2 = np.zeros((128, 2), np.float32)
    ones2[0:64, 0] = 1.0
    ones2[64:128, 1] = 1.0
    e2f_host = np.zeros((8, 128), np.float32)
    for _b in range(4):
        e2f_host[2 * _b, 0:64] = 1.0
        e2f_host[2 * _b + 1, 64:128] = 1.0

    Wq, Wk, Wv, Wo = (qkvo_w[0:D], qkvo_w[D:2 * D], qkvo_w[2 * D:3 * D],
                      qkvo_w[3 * D:4 * D])

    in_maps = []
    for core in range(8):
        b = core // HGROUPS
        hg = core % HGROUPS
        heads = list(range(hg * NHEADS, (hg + 1) * NHEADS))
        perm = np.r_[0:16, 32:48, 16:32, 48:64]
        blocks = []
        for h in heads:
            blocks.append(lam0 * Wq[h * HD:(h + 1) * HD][perm].T)
            blocks.append(lam0 * Wk[h * HD:(h + 1) * HD][perm].T)
        wqk = np.concatenate(blocks, axis=1).astype(np.float32)
        wqk = np.ascontiguousarray(
            wqk.reshape(NCHUNK, 128, 512).transpose(1, 0, 2)
            .reshape(128, -1)).astype(ml_dtypes.bfloat16)
        wv_cols = [lam0 * Wv[h * HD:(h + 1) * HD].T for h in heads]
        wv_cols.append(ve_gate_w[heads].T)
        wv = np.concatenate(wv_cols, axis=1).astype(np.float32)
        wv = np.ascontiguousarray(
            wv.reshape(NCHUNK, 128, 260).transpose(1, 0, 2)
            .reshape(128, -1)).astype(ml_dtypes.bfloat16)
        wga = np.zeros((D, 34), np.float32)
        wga[:, [0, 1, 32, 33]] = attn_gate_w[heads].T
        wga = np.ascontiguousarray(
            wga.reshape(NCHUNK, 128, 34).transpose(1, 0, 2)
            .reshape(128, -1)).astype(ml_dtypes.bfloat16)
        wo = (lam1 * Wo[:, hg * 256:(hg + 1) * 256].T).astype(np.float32)
        wo = np.ascontiguousarray(
            wo.reshape(2, 128, 1024).transpose(1, 0, 2).reshape(128, -1))
        xTn = x[b].T.astype(np.float32)  # [D, T]
        # [p, (tau c t)] layout: per-tau contiguous 16KB rows
        xT = np.ascontiguousarray(
            xTn.reshape(NCHUNK, 128, NTT, TTILE).transpose(1, 2, 0, 3)
            .reshape(128, -1)).astype(ml_dtypes.bfloat16)
        ve2 = np.ascontiguousarray(
            VE_GATE_SCALE * ve[b, :, hg * 256:(hg + 1) * 256],
            dtype=np.float32).astype(ml_dtypes.bfloat16)
        in_maps.append({
            "xT": xT, "ve2": ve2, "wqk": wqk, "wv": wv, "wga": wga,
            "wo": wo, "cdup": cdup, "s2dup": s2dup, "qaug": qaug,
            "kaug": kaug, "ones2": ones2,
            "e2f": e2f_host,
        })

    global LAST_RESULT
    res = run_bass_kernel_spmd(nc, in_maps, list(range(8)), trace=TRACE)
    LAST_RESULT = res
    out = np.zeros((B, T, D), dtype=np.float32)
    for core in range(8):
        out[core // HGROUPS] += res.results[core]["out"]
    return out

